# revision 29
# baseline (speedup 1.0000x reference)
"""Trainium2 Bass kernel for MultiHeadHypergraphAttention.

Problem: queries (4, 1024, 512), keys (4, 4096, 512), incidence (4, 1024, 4096) i32,
torch-Linear Q/K/V/O projections, per-head masked softmax attention.

Sharding (8 cores): batch (4) x head-group (2 groups of 4 heads).
Core c handles batch b = c//2, head group g = c%2 and produces the partial
output projection for its 4 heads; the host sums the two partials per batch.

Device-side layout ("scores transposed"): S^T is computed with nodes on
partitions and edges on the free axis, so the incidence mask (host-transposed
to (nodes, edges)) is applied in its natural layout and attention weights P^T
feed the attn@V matmul directly as the moving operand (V' stationary), which
produces O^T (head dims on partitions) — exactly the orientation the output
projection needs, so no on-chip transposes at all.

Softmax normalization is folded into the output: V is augmented with a
ones-column so attn@V also produces row sums; O^T rows are divided by those
sums (reciprocal broadcast across partitions via small DRAM bounces).
Masked entries are killed by multiplying exp(s/8) with the 0/1 mask (bf16).

v2 changes vs the first working version:
- All streaming inputs are pre-cast to bf16 on the host (qT 1MB, kT 4MB,
  mask 8MB per core instead of 2/8/16MB f32/i32), and all weights are packed
  into one bf16 "wall" tensor loaded with a single DMA. Loads are split
  across the HWDGE (sync) and SWDGE (gpsimd) queues so descriptor issue
  doesn't serialize.
- bk is dropped entirely: adding K's bias shifts every score of an edge by
  the same constant, which cancels in the softmax (exact).
- O^T tiles are evacuated from PSUM to SBUF (bf16) right when a head's
  accumulation finishes, so the 2-deep PSUM rotation never stalls the next
  head's attn@V behind the (DMA-latency-bound) normalization chain.
- The tail is tightened: head 3's normalization overlaps output-projection
  warm work, and the output is written bf16 (host sums partials in f32).
- A dummy exp preloads the ACT table set during the DMA ramp, and a burst of
  throwaway matmuls warms the PE HAM clock gate before real work arrives.

v3 changes vs v2:
- Critical-path DMAs (weights sans Wo, queries, kt windows 0/1, mask chunks
  0-3) all ride the sync ring so the first scores issue ~8us in; Q-proj is
  c-outer so accumulation starts as chunks land.
- Head-2 scores start inside phase 1's tail (ACT never idles at the seam).
- Output projection split into contraction quarters (pair0 / head2 / head3)
  so only 8 matmuls wait on head 3's norm chain; the rest hide under it.
"""

import sys
import os

for _p in ("/opt/trn_rl_repo",):
    if _p not in sys.path and os.path.isdir(_p):
        sys.path.insert(0, _p)

import numpy as np
import ml_dtypes
from contextlib import ExitStack

import concourse.bass as bass
import concourse.mybir as mybir
import concourse.tile as tile
from concourse import bacc
from concourse.bass_utils import run_bass_kernel_spmd

BF16 = mybir.dt.bfloat16
F32 = mybir.dt.float32

BS, E, N, D = 4, 1024, 4096, 512
HL = 4                   # heads per core (local)
NCHUNK = N // 128        # 32
ECHUNK = E // 128        # 8
WBLK = 772               # per-c weight block in the wall: 256 wq + 256 wk + 260 wv
WO_OFF = 4 * WBLK        # 3088

NP_BF16 = ml_dtypes.bfloat16

LAST_EXEC_TIME_NS = None
_CACHED_NC = None


def _build_nc():
    nc = bacc.Bacc("TRN2", target_bir_lowering=False, debug=False, num_devices=8)

    qT_d = nc.dram_tensor("qT", (4, 128, E), BF16, kind="ExternalInput").ap()
    kT_d = nc.dram_tensor("kT", (8, 128, 4, 512), BF16, kind="ExternalInput").ap()
    mT_d = nc.dram_tensor("mT", (N, E), BF16, kind="ExternalInput").ap()
    wall_d = nc.dram_tensor("wall", (128, WO_OFF + 1024), BF16, kind="ExternalInput").ap()
    bq_d = nc.dram_tensor("bq2", (2, 128, 1), F32, kind="ExternalInput").ap()
    out_d = nc.dram_tensor("out", (E, 512), BF16, kind="ExternalOutput").ap()

    with tile.TileContext(nc) as tc, ExitStack() as ctx:
        persist = ctx.enter_context(tc.tile_pool(name="persist", bufs=1))
        work = ctx.enter_context(tc.tile_pool(name="work", bufs=1))
        ps = ctx.enter_context(tc.tile_pool(name="ps", bufs=1, space="PSUM"))
        dpool = ctx.enter_context(tc.tile_pool(name="dpool", bufs=1, space="DRAM"))

        # ---------------- constants ----------------

        # Q~T zero halves and V' ones columns never change: write them first
        # on the vector engine so nothing queues behind DMA launches
        QTs = [persist.tile([128, E], BF16, tag=f"QTs{l}", name=f"QTs{l}")
               for l in range(HL)]
        for l in range(HL):
            r = l % 2
            zsl = slice(64 * (1 - r), 64 * (1 - r) + 64)
            nc.vector.memset(QTs[l][zsl, :], 0.0)
        Vs = persist.tile([128, NCHUNK * 260], BF16, tag="Vs")
        ones_cols = Vs.rearrange("p (n h c) -> p n h c", n=NCHUNK, h=4)[:, :, :, 64:65]
        nc.vector.memset(ones_cols, 1.0)

        # preload the exp table set while the first DMAs are in flight
        act_dummy = work.tile([1, 16], F32, tag="actd", name="act_dummy")
        nc.scalar.activation(act_dummy, QTs[0][64:65, 0:16],
                             mybir.ActivationFunctionType.Exp, bias=0.0, scale=1.0)

        # ------------- bulk loads (all bf16, pre-cast on host) -------------
        # v3: the critical-path loads (weights sans Wo, queries, kt windows
        # 0/1, mask chunks 0-3) all ride the sync (HWDGE) ring so the first
        # scores issue ~8us in; everything later rides SWDGE, whose ~9us
        # boot hides under the sync stream.
        wall = persist.tile([128, WO_OFF + 1024], BF16, tag="wall")
        nc.sync.dma_start(out=wall[:, 0:WO_OFF], in_=wall_d[:, 0:WO_OFF])

        def wq_ap(c):
            return wall[:, c * WBLK: c * WBLK + 256]

        def wk_ap(c):
            return wall[:, c * WBLK + 256: c * WBLK + 512]

        def wv_ap(c):
            return wall[:, c * WBLK + 512: c * WBLK + 772]

        def wo_ap(p):
            return wall[:, WO_OFF + p * 512: WO_OFF + (p + 1) * 512]

        qTall = persist.tile([128, 4 * E], BF16, tag="qTall")
        qt_v = qTall.rearrange("p (c e) -> p c e", c=4)
        for c in (0, 1):
            nc.sync.dma_start(out=qt_v[:, c, :], in_=qT_d[c])

        bqs = []
        for p in range(2):
            bq_t = persist.tile([128, 1], F32, tag=f"bq{p}", name=f"bq{p}")
            nc.sync.dma_start(out=bq_t, in_=bq_d[p])
            bqs.append(bq_t)

        kTall = persist.tile([128, 4 * 8 * 512], BF16, tag="kTall")
        kt_v = kTall.rearrange("p (c w j) -> p c w j", c=4, w=8)
        Mb = persist.tile([128, NCHUNK * E], BF16, tag="Mb")

        def mask_load(nn, eng):
            src = mT_d[nn * 256:(nn + 1) * 256, :].rearrange(
                "(two p) e -> p two e", p=128)
            dst = Mb[:, 2 * nn * E:(2 * nn + 2) * E].rearrange(
                "p (two e) -> p two e", two=2)
            eng.dma_start(out=dst, in_=src)

        # SWDGE (cheap ~0.8us launches, ~1.9us/0.5MB item): qt chunks 2/3
        # and kt window 0 lead so Q-proj/K-proj start ~14us in; mask chunks
        # pace the b1 stream with growing margin; Wo rides mid-stream.
        nc.gpsimd.dma_start(out=qt_v[:, 2, :], in_=qT_d[2])
        nc.gpsimd.dma_start(out=kt_v[:, :, 0, :], in_=kT_d[0])
        nc.gpsimd.dma_start(out=qt_v[:, 3, :], in_=qT_d[3])
        mask_load(0, nc.gpsimd)
        nc.gpsimd.dma_start(out=kt_v[:, :, 1, :], in_=kT_d[1])
        mask_load(1, nc.gpsimd)
        nc.gpsimd.dma_start(out=kt_v[:, :, 2, :], in_=kT_d[2])
        mask_load(2, nc.gpsimd)
        mask_load(3, nc.gpsimd)
        for w in range(3, 8):
            nc.gpsimd.dma_start(out=kt_v[:, :, w, :], in_=kT_d[w])
            mask_load(2 * w - 2, nc.gpsimd)
            mask_load(2 * w - 1, nc.gpsimd)
            if w == 4:
                nc.gpsimd.dma_start(out=wall[:, WO_OFF:],
                                    in_=wall_d[:, WO_OFF:])
        mask_load(14, nc.gpsimd)
        mask_load(15, nc.gpsimd)

        # ------------- PE warm-up: throwaway matmuls on the zero half ------
        # keeps the HAM activity window busy during the DMA ramp so the real
        # projections run at 2.4 GHz instead of 1.2 GHz
        def pe_warm(k):
            for _ in range(k):
                wt = ps.tile([128, 256], F32, tag="st", bufs=2, name="warm")
                nc.tensor.matmul(wt, QTs[0][64:128, 0:128],
                                 QTs[0][64:128, 0:256], start=True, stop=True,
                                 skip_group_check=True)

        # just enough to bridge engine start (~8us) to the first qt arrival
        # (~10.5us); a big burst would block the queue ahead of Q-proj
        pe_warm(10)

        # ---------------- Q projection ----------------
        # Q~T[l] (128, 1024) bf16: rows [64r, 64r+64) = head l's Q^T, rest 0
        # (l = 2p + r), so scores matmuls contract over the full 128
        # partitions (1 cyc/row) against KTs[p]. c-outer ordering lets each
        # accumulation step start as soon as its qT chunk lands.
        qps = [ps.tile([128, E], F32, tag="st", bufs=2, name=f"qp{p}")
               for p in range(2)]
        for ci, c in enumerate((2, 0, 1, 3)):
            for p in range(2):
                for e2 in range(2):
                    nc.tensor.matmul(
                        qps[p][:, e2 * 512:(e2 + 1) * 512],
                        wq_ap(c)[:, p * 128:(p + 1) * 128],
                        qt_v[:, c, e2 * 512:(e2 + 1) * 512],
                        start=(ci == 0), stop=(ci == 3))
        # bias adds split across the (idle) scalar engine for pair 0 -- the
        # fast path to the first exp -- and the vector engine for pair 1
        for p in range(2):
            for r in range(2):
                sl = slice(64 * r, 64 * r + 64)
                if p == 0:
                    nc.scalar.activation(QTs[r][sl, :], qps[0][sl, :],
                                         mybir.ActivationFunctionType.Identity,
                                         bias=bqs[0][sl, :], scale=1.0)
                else:
                    nc.vector.tensor_scalar_add(QTs[2 + r][sl, :],
                                                qps[1][sl, :], bqs[1][sl, :])

        # ------------- K/V projections merged with attention ---------------
        KTs = [persist.tile([128, N], BF16, tag=f"KTs{p}", name=f"KTs{p}")
               for p in range(2)]
        pairN = [persist.tile([128, E], BF16, tag=f"pairN{p}", name=f"pairN{p}")
                 for p in range(2)]
        oTs = {}
        oT_sb = {}
        Ps = {}

        def score_one(l, n, share_kblk=False):
            # scores + exp + mask for head l, node chunk n -> P^T in Ps.
            # Contraction runs over the full 128 partitions (the other
            # head's 64 rows of Q~T are zero): a 64-deep contraction would
            # free PE time, but an idle-ish PE trips the HAM clock gate
            # down to 1.2 GHz, which costs more than the saved cycles.
            # ldweights=False on matmuls whose stationary is already
            # resident (same kblk as the immediately preceding PE matmul)
            # skips the redundant ~110ns weight reload.
            p = l // 2
            kblk = KTs[p][:, n * 128:(n + 1) * 128]
            st = ps.tile([128, E], F32, tag="st", bufs=2, name=f"st{l}_{n}")
            for e2 in range(2):
                sl = slice(e2 * 512, (e2 + 1) * 512)
                nc.tensor.matmul(st[:, sl], kblk, QTs[l][:, sl],
                                 start=True, stop=True)
            Praw = work.tile([128, E], BF16, tag="Praw", bufs=5,
                             name=f"Praw{l}_{n}")
            nc.scalar.activation(Praw, st, mybir.ActivationFunctionType.Exp,
                                 bias=0.0, scale=0.125)
            P = work.tile([128, E], BF16, tag="P", bufs=5, name=f"P{l}_{n}")
            nc.vector.tensor_mul(P, Praw, Mb[:, n * E:(n + 1) * E])
            Ps[(l, n)] = P

        def score_pair(p, n):
            score_one(2 * p, n)
            score_one(2 * p + 1, n, share_kblk=True)

        def av_part(l, n):
            # attn @ V' for (head l, node chunk n), accumulating into oTs[l]
            P = Ps.pop((l, n))
            vblk = Vs[:, n * 260 + l * 65:n * 260 + l * 65 + 65]
            for e2 in range(2):
                sl = slice(e2 * 512, (e2 + 1) * 512)
                nc.tensor.matmul(oTs[l][:, sl], vblk, P[:, sl],
                                 start=(n == 0), stop=(n == NCHUNK - 1))

        norm_state = {}

        def evac(l):
            # pull the finished head out of PSUM in ONE 65-row bf16 copy:
            # rows 0-63 = O'^T, row 64 = the softmax sums (bf16 sums cost
            # ~0.3% relative error on Z, far inside the tolerance, and save
            # a separate 1.2us single-lane f32 copy per head).
            ot = persist.tile([65, E], BF16, tag=f"oTsb{l}", name=f"oTsb{l}")
            nc.vector.tensor_copy(ot, oTs[l][0:65, :])
            norm_state[("sums", l)] = ot[64:65, :]
            oT_sb[l] = ot[0:64, :]

        def norm_stage1(l, eng=None):
            # bounce the (bf16) sums row to DRAM, reshaped (64, 16) so the
            # iterative reciprocal runs 64 lanes wide
            eng = eng or nc.sync
            sums = norm_state.pop(("sums", l))
            sums_d = dpool.tile([1, E], BF16, tag="sums_d", bufs=2,
                                name=f"sums_d{l}")
            eng.dma_start(out=sums_d, in_=sums)
            sums64 = work.tile([64, 16], BF16, tag="sums64", bufs=2,
                               name=f"sums64{l}")
            eng.dma_start(
                out=sums64, in_=sums_d.rearrange("one (p k) -> (one p) k", p=64))
            norm_state[l] = sums64

        def norm_stage2(l, eng=None):
            eng = eng or nc.sync
            sums64 = norm_state.pop(l)
            recip64 = work.tile([64, 16], F32, tag="recip64", bufs=2,
                                name=f"recip64{l}")
            nc.vector.reciprocal(recip64, sums64)
            rec16 = work.tile([64, 16], BF16, tag="rec16", bufs=2,
                              name=f"rec16{l}")
            nc.vector.tensor_copy(rec16, recip64)
            rec_d = dpool.tile([64, 16], BF16, tag="rec_d", bufs=2,
                               name=f"rec_d{l}")
            eng.dma_start(out=rec_d, in_=rec16)
            rec_row = rec_d.rearrange("p k -> (p k)").unsqueeze(0)
            recb = work.tile([64, E], BF16, tag="recb", bufs=2, name=f"recb{l}")
            eng.dma_start(out=recb, in_=rec_row.to_broadcast((64, E)))
            norm_state[l] = recb

        def norm_stage3(l):
            # divide O'^T head rows by the exp-sum row (broadcast reciprocal)
            p, r = l // 2, l % 2
            recb = norm_state.pop(l)
            nc.vector.tensor_mul(pairN[p][64 * r:64 * r + 64, :],
                                 oT_sb[l], recb)

        def proj_k(w, p):
            kp = ps.tile([128, 512], F32, tag="st", bufs=2, name=f"kp{p}_{w}")
            for c in range(4):
                nc.tensor.matmul(
                    kp, wk_ap(c)[:, p * 128:(p + 1) * 128], kt_v[:, c, w, :],
                    start=(c == 0), stop=(c == 3))
            nc.vector.tensor_copy(KTs[p][:, w * 512:(w + 1) * 512], kp)

        def proj_v(n):
            w, t = n // 4, n % 4
            vp = ps.tile([128, 260], F32, tag="st", bufs=2, name=f"vp{n}")
            for c in range(4):
                nc.tensor.matmul(vp, kt_v[:, c, w, t * 128:(t + 1) * 128],
                                 wv_ap(c), start=(c == 0), stop=(c == 3))
            sub = Vs[:, n * 260:(n + 1) * 260].rearrange(
                "p (h c) -> p h c", h=4)[:, :, 0:64]
            vsub = vp.rearrange("p (h c) -> p h c", h=4)[:, :, 0:64]
            nc.vector.tensor_copy(sub, vsub)

        for l in (0, 1):
            oTs[l] = ps.tile([65, E], F32, tag="outT", bufs=2, name=f"oT{l}")

        # merged pipeline: heads 0/1 attention lags the K/V projections by
        # one window so DMA-arrival jitter is absorbed by the persistent
        # Mb/KTs/Vs tiles. attn@V lags the scores by one chunk so the PE
        # never waits on exp/mask.
        def b1_chunk(n):
            # issue order matters: the PE queue is in-order, and score(1,n)
            # stalls on its PSUM slot until exp(1,n-1) completes -- so the
            # (ready) av matmuls go between the two score_ones
            score_one(0, n)
            if n > 0:
                av_part(0, n - 1)
            score_one(1, n)
            if n > 0:
                av_part(1, n - 1)

        # v4: proj_k(w, 1) feeds only heads 2/3 (phase 2), so it moves out of
        # phase 1 entirely -- phase 1 was PE-bound, phase 2 has PE slack.
        b1_next = 0
        for w in range(8):
            steps = [lambda w=w: proj_k(w, 0)] + \
                    [lambda n=n: proj_v(n) for n in range(4 * w, 4 * w + 4)]
            for i, step in enumerate(steps):
                if (w > 0 and i in (0, 1, 2, 4)) or (w == 0 and i in (1, 2)):
                    b1_chunk(b1_next)
                    b1_next += 1
                step()
        while b1_next < NCHUNK:
            b1_chunk(b1_next)
            proj_k(b1_next - 30, 1)
            score_one(2, b1_next - 30)
            b1_next += 1
        # seam: head-2's score stream is already 2 chunks deep (buffered in
        # the P ring) so the ACT engine never dips across the boundary
        av_part(0, NCHUNK - 1)
        evac(0)
        norm_stage1(0)
        score_one(2, 2)
        av_part(1, NCHUNK - 1)
        evac(1)
        norm_stage1(1)
        score_one(2, 3)

        # heads 2 and 3 interleaved, head 3 staggered SG chunks behind: the
        # solo head-3 steps at the end still feed ACT one exp per step (so
        # the attention end time is unchanged), but head 2's entire
        # normalization chain completes ~SG steps early, leaving only head
        # 3's chain exposed in the tail. Remaining proj_k(w,1) windows ride
        # phase 2's PE slack, 4+ chunks ahead of their first consumer.
        SG = 10
        for l in (2, 3):
            oTs[l] = ps.tile([65, E], F32, tag="outT", bufs=2, name=f"oT{l}")
        av_part(2, 0)
        av_part(2, 1)
        plan = {5: ("s2", 0), 7: ("s3", 0),
                9: ("s2", 1), 11: ("s3", 1)}
        stage_fn = {"s1": norm_stage1, "s2": norm_stage2, "s3": norm_stage3}
        for t in range(4, NCHUNK + SG):
            if t < NCHUNK:
                score_one(2, t)
            if t - 2 < NCHUNK:
                av_part(2, t - 2)
            if t in (4, 8, 12, 16, 20, 24):
                proj_k((t - 4) // 4 + 2, 1)
            if SG <= t:
                score_one(3, t - SG)
            if SG + 1 <= t:
                av_part(3, t - SG - 1)
            if t in plan:
                op, hl = plan[t]
                stage_fn[op](hl)
            if t == NCHUNK + 2:
                evac(2)
                norm_stage1(2)
            if t == NCHUNK + 3:
                norm_stage2(2)
            if t == NCHUNK + 4:
                norm_stage3(2)
        av_part(3, NCHUNK - 1)

        # head-3 evacuation on the (now idle) scalar engine; its norm
        # chain rides the sync queue while the q0/q2 output-projection
        # matmuls fill the chain's DMA latency
        ot3 = persist.tile([65, E], BF16, tag="oTsb3", name="oTsb3")
        nc.scalar.copy(ot3, oTs[3][0:65, :])
        norm_state[("sums", 3)] = ot3[64:65, :]
        oT_sb[3] = ot3[0:64, :]
        norm_stage1(3)

        # ---------------- tail: output projection ------------
        # quarters: q0 = pairN[0] (heads 0/1), q2 = head 2, q3 = head 3.
        fpairs = [ps.tile([128, E], F32, tag=("st" if jp < 2 else "outT"),
                          bufs=2, name=f"fpair{jp}") for jp in range(4)]

        def f_ap(j):
            return fpairs[j // 2][:, (j % 2) * 512:(j % 2) * 512 + 512]

        def fq(j, q):
            eb = slice(j * 128, (j + 1) * 128)
            if q == 0:
                nc.tensor.matmul(f_ap(j), pairN[0][:, eb], wo_ap(0),
                                 start=True, stop=False)
            elif q == 2:
                nc.tensor.matmul(f_ap(j), pairN[1][0:64, eb],
                                 wo_ap(1)[0:64, :], start=False, stop=False)
            else:
                nc.tensor.matmul(f_ap(j), pairN[1][64:128, eb],
                                 wo_ap(1)[64:128, :], start=False, stop=True)

        foall = persist.tile([128, ECHUNK * 512], BF16, tag="foall")
        out_v = out_d.rearrange("(h e p) d -> h p e d", h=2, p=128)
        fo_v = foall.rearrange("p (h e d) -> h p e d", h=2, e=4)

        def f_close(j):
            fq(j, 3)
            # final copies on the scalar engine -- the DVE queue is
            # otherwise the tail's serial bottleneck
            nc.scalar.copy(foall[:, j * 512:(j + 1) * 512], f_ap(j))
            if j in (1, 3, 5):
                q = j // 2
                nc.sync.dma_start(
                    out=out_v[q // 2][:, (q % 2) * 2:(q % 2) * 2 + 2, :],
                    in_=fo_v[q // 2][:, (q % 2) * 2:(q % 2) * 2 + 2, :])
            elif j >= 6:
                e8 = j - 4
                nc.sync.dma_start(out=out_v[1][:, e8:e8 + 1, :],
                                  in_=fo_v[1][:, e8:e8 + 1, :])

        for j in range(ECHUNK):
            fq(j, 0)
        norm_stage2(3)
        for j in range(ECHUNK):
            fq(j, 2)
        norm_stage3(3)
        for j in range(ECHUNK):
            f_close(j)

    nc.compile()
    return nc


def _get_nc():
    global _CACHED_NC
    if _CACHED_NC is None:
        _CACHED_NC = _build_nc()
    return _CACHED_NC


def _make_in_maps(queries, keys, incidence_matrix, Wq, bq, Wk, bk, Wv, bv, Wo, bo):
    """Host-side sharding + layout marshalling (transposes + bf16 casts).

    bk is intentionally dropped: K's bias adds the same constant to every
    score of an edge, which the per-edge softmax normalization cancels.
    """
    queries = np.asarray(queries, dtype=np.float32)
    keys = np.asarray(keys, dtype=np.float32)
    incidence = np.asarray(incidence_matrix, dtype=np.int32)
    Wq = np.asarray(Wq, dtype=np.float32)
    Wk = np.asarray(Wk, dtype=np.float32)
    Wv = np.asarray(Wv, dtype=np.float32)
    Wo = np.asarray(Wo, dtype=np.float32)
    bq = np.asarray(bq, dtype=np.float32)

    qT_b, kT_b, mT_b = [], [], []
    for b in range(BS):
        qT = queries[b].T.astype(NP_BF16)                       # (512, 1024)
        qT_b.append(np.ascontiguousarray(qT.reshape(4, 128, E)))
        kT = keys[b].T.astype(NP_BF16)                          # (512, 4096)
        kT_b.append(np.ascontiguousarray(
            kT.reshape(4, 128, 8, 512).transpose(2, 1, 0, 3)))  # (8,128,4,512)
        mT_b.append(np.ascontiguousarray(incidence[b].T.astype(NP_BF16)))

    in_maps = []
    for core in range(8):
        b, g = core // 2, core % 2
        sl = slice(g * 256, (g + 1) * 256)
        wall = np.zeros((128, WO_OFF + 1024), np.float32)
        wqT = Wq[sl, :].T                                       # (512, 256)
        wkT = Wk[sl, :].T
        wvT = np.zeros((D, 260), np.float32)
        for l in range(HL):
            rows = slice(g * 256 + l * 64, g * 256 + l * 64 + 64)
            wvT[:, l * 65:l * 65 + 64] = Wv[rows, :].T
        woT = Wo[:, sl].T                                       # (256, 512)
        for c in range(4):
            rs = slice(c * 128, (c + 1) * 128)
            wall[:, c * WBLK: c * WBLK + 256] = wqT[rs]
            wall[:, c * WBLK + 256: c * WBLK + 512] = wkT[rs]
            wall[:, c * WBLK + 512: c * WBLK + 772] = wvT[rs]
        for p in range(2):
            wall[:, WO_OFF + p * 512: WO_OFF + (p + 1) * 512] = \
                woT[p * 128:(p + 1) * 128]
        in_maps.append({
            "qT": qT_b[b],
            "kT": kT_b[b],
            "mT": mT_b[b],
            "wall": wall.astype(NP_BF16),
            "bq2": bq[sl].reshape(2, 128, 1).copy(),
        })
    return in_maps


def kernel(**inputs):
    global LAST_EXEC_TIME_NS
    nc = _get_nc()
    in_maps = _make_in_maps(**inputs)
    trace = bool(os.environ.get("BASS_TRACE"))
    if trace:
        _install_ntff_hook()
    res = run_bass_kernel_spmd(nc, in_maps, core_ids=list(range(8)), trace=trace)
    LAST_EXEC_TIME_NS = res.exec_time_ns
    out = np.zeros((BS, E, D), np.float32)
    # attention rows sum to 1, so the V bias contributes bv @ Wo.T exactly;
    # add it (and bo) once here instead of on the device
    bias = (np.asarray(inputs["bo"], np.float32)
            + np.asarray(inputs["bv"], np.float32)
            @ np.asarray(inputs["Wo"], np.float32).T)
    for b in range(BS):
        out[b] = (res.results[2 * b]["out"].astype(np.float32)
                  + res.results[2 * b + 1]["out"].astype(np.float32) + bias)
    return out


def _install_ntff_hook():
    """Recreate the missing antenv.axon_hooks glue so trace=True captures NTFF."""
    import types
    if "antenv.axon_hooks" in sys.modules:
        return
    try:
        from trn_agent_boot.trn_boot import _ntff_profile_via_ctypes
        hook = _ntff_profile_via_ctypes("/opt/axon/libaxon_pjrt.so")
        m = types.ModuleType("antenv.axon_hooks")
        m.get_axon_ntff_profile_hook = lambda: hook
        m.set_axon_ntff_profile_hook = lambda h: None
        sys.modules["antenv.axon_hooks"] = m
    except Exception:
        pass

